# revision 1
# baseline (speedup 1.0000x reference)
"""Deformable 3D conv (nn_DeformableConv3D) Trainium2 Bass kernel.

Strategy (per core, 8 cores, sharded over output z in blocks of 4):
  1. offset/mask 3x3x3 convs as implicit GEMM: contraction packed (ky,c)=96,
     9 matmuls per 512-voxel chunk accumulating in PSUM.
  2. Per 128-voxel tile, transpose offset/mask fields to [n, ch] layout,
     compute trilinear floors/fracs/corner-weights and int32 gather indices
     on DVE/ACT.
  3. Build a DRAM staging buffer of (z,y)-corner-packed fp16 rows
     (row m=(z0,y0,x0) holds [c=32][dzy=4]; x-corner pair = adjacent rows),
     gather 512B elems (8 corners x 32ch) per (tap, voxel) sample with
     indirect DMA.
  4. Scale by (mask * trilinear) weights + fold corners on DVE; last corner
     pair folds into the main einsum contraction.
  5. Main einsum as 14 matmul chunks of contraction (2 taps x 32c x 2 pair),
     PE transposes to get [contraction, n] tiles.
"""
import os
import sys
import functools

import numpy as np

for _p in ("/opt/trn_rl_repo", "/root/.axon_site/_ro/trn_rl_repo"):
    if os.path.isdir(_p) and _p not in sys.path:
        sys.path.insert(0, _p)

from concourse import bass, bacc, mybir  # noqa: E402
import concourse.tile as tile  # noqa: E402
from concourse.bass_utils import run_bass_kernel_spmd  # noqa: E402

FP32 = mybir.dt.float32
FP16 = mybir.dt.float16
I32 = mybir.dt.int32
I16 = mybir.dt.int16
AF = mybir.ActivationFunctionType
OP = mybir.AluOpType

# problem constants
C, D, H, W = 32, 32, 64, 64
KS, K, COUT = 3, 27, 32
NCORES = 8
ZB = D // NCORES                # output z per core (4)
N_CORE = ZB * H * W             # 16384 voxels per core
NT = N_CORE // 128              # 128 n-tiles per core
GRP = 8                         # n-tiles per field group
NG = NT // GRP                  # 16 groups

# staging geometry: slab-local padded coords
# pos_z_local = nz_local + kz + 2 + off   (in [0.15, 8.85])
# pos_y_local = ny + ky + 3 + off         (in [1.15, 69.85])
# pos_x_local = nx + kx + 3 + off
SZ = 10                         # z slices in x slab (z in [4c-3, 4c+7))
RY = H + 7                      # 71 y0 rows
RX = W + 8                      # 72 x values per row
ROWS_PER_Z = RY * RX            # 5112
NROWS = SZ * ROWS_PER_Z         # 51120 staging rows
ROW_EL = C * 4                  # 128 fp16 per row (c x dzy)

# conv slab geometry: z in [4c-1, 4c+5) -> 6 slices, x margin 1 -> 66
CZ, CY, CX = 6, H, W + 2

# magic constants for floor-by-round-to-nearest (fp32)
MAGIC = 12582912.0              # 1.5 * 2**23

FLD = 128                       # 0:81 offsets, 96:123 mask, rest pad
MB = 96                         # mask block start
NCH = 27                        # einsum chunks: (2dx, 32c, 2dy) per tap


def build_module(debug_outs=False):
    nc = bacc.Bacc("TRN2", target_bir_lowering=False, debug=False)

    # ---- dram I/O -------------------------------------------------------
    xs_conv = nc.dram_tensor("xs_conv", [96, CZ * CY * CX], FP16,
                             kind="ExternalInput")
    base_c = nc.dram_tensor("base_c", [128, NT, 81], FP16, kind="ExternalInput")
    wconv = nc.dram_tensor("wconv", [9, 96, FLD], FP16, kind="ExternalInput")
    weins = nc.dram_tensor("weins", [NCH, 128, COUT], FP16, kind="ExternalInput")
    cbias = nc.dram_tensor("cbias", [FLD, 1], FP32, kind="ExternalInput")
    obias = nc.dram_tensor("obias", [COUT, 1], FP32, kind="ExternalInput")
    ident = nc.dram_tensor("ident", [128, 128], FP16, kind="ExternalInput")
    out_s = nc.dram_tensor("out_s", [COUT, N_CORE], FP32, kind="ExternalOutput")
    stag = nc.dram_tensor("stag", [NROWS, ROW_EL], FP16, kind="ExternalInput")

    dbg = {}
    if debug_outs:
        dbg["offmask"] = nc.dram_tensor("d_offmask", [N_CORE, FLD], FP16,
                                        kind="ExternalOutput")
        dbg["idx"] = nc.dram_tensor("d_idx", [N_CORE, K], I16,
                                    kind="ExternalOutput")
        dbg["s"] = nc.dram_tensor("d_s", [N_CORE, K * 8], FP16,
                                  kind="ExternalOutput")
        dbg["val"] = nc.dram_tensor("d_val", [128, K * 64], FP16,
                                    kind="ExternalOutput")

    with tile.TileContext(nc) as tc:
        build_kernel(nc, tc, xs_conv, base_c, wconv, weins, cbias, obias,
                     ident, out_s, stag, dbg)
    nc.compile()
    return nc


def build_kernel(nc, tc, xs_conv, base_c, wconv, weins, cbias, obias,
                 ident, out_s, stag, dbg):
    with tc.tile_pool(name="konst", bufs=1) as kp:
        id_t = kp.tile([128, 128], FP16)
        nc.sync.dma_start(id_t[:], ident[:])
        cb_t = kp.tile([FLD, 1], FP32)
        nc.sync.dma_start(cb_t[:], cbias[:])
        ob_t = kp.tile([COUT, 1], FP32)
        nc.sync.dma_start(ob_t[:], obias[:])
        wc_t = kp.tile([96, 9 * FLD], FP16, tag="wc")
        for t9 in range(9):
            nc.sync.dma_start(wc_t[:, t9 * FLD:(t9 + 1) * FLD],
                              wconv[t9, :, :])
        we_t = kp.tile([128, NCH * COUT], FP16, tag="we")
        for q in range(NCH):
            nc.sync.dma_start(we_t[:, q * COUT:(q + 1) * COUT],
                              weins[q, :, :])

        with tc.tile_pool(name="xr", bufs=1) as xrp:
            xr = xrp.tile([96, CZ * CY * CX], FP16)  # (ky,c) x (z,y,x)
            nc.sync.dma_start(xr[:], xs_conv[:])
            xr_v = xr[:].rearrange("p (z y x) -> p z y x", z=CZ, y=CY, x=CX)
            main_loop(nc, tc, xr_v, base_c, wc_t, we_t, cb_t, ob_t, id_t,
                      stag, out_s, dbg)


def main_loop(nc, tc, xr_v, base_c, wc_t, we_t, cb_t, ob_t, id_t, stag,
              out_s, dbg):
    with (
        tc.tile_pool(name="cps", bufs=2, space="PSUM") as conv_pp,
        tc.tile_pool(name="ot", bufs=2) as ot_p,
        tc.tile_pool(name="otps", bufs=2, space="PSUM") as ot_pp,
        tc.tile_pool(name="fld", bufs=2) as fp_,
        tc.tile_pool(name="idxd", bufs=2, space="DRAM") as idp,
        tc.tile_pool(name="gat", bufs=2) as gp,
        tc.tile_pool(name="scl", bufs=2) as scp,
        tc.tile_pool(name="fold", bufs=2) as dp,
        tc.tile_pool(name="tps", bufs=2, space="PSUM") as tp_pp,
        tc.tile_pool(name="rhs", bufs=4) as rp,
        tc.tile_pool(name="ops", bufs=2, space="PSUM") as o_pp,
        tc.tile_pool(name="osb", bufs=2) as ob_p,
    ):
        for grp in range(NG):
            # ---- conv: 2 chunks of 512 voxels --------------------------
            off8 = fp_.tile([128, GRP * FLD], FP16, tag="off8")
            for cc in range(2):
                chunk = grp * 2 + cc          # global 512-chunk id
                zl = chunk // 8               # local out z
                y0 = (chunk % 8) * 8          # first of 8 y rows
                cps = conv_pp.tile([FLD, 512], FP32, tag="cpsum")
                for t9 in range(9):
                    kz, kx = t9 // 3, t9 % 3
                    rhs = xr_v[:, zl + kz, y0:y0 + 8, kx:kx + W]
                    nc.tensor.matmul(
                        cps[:], wc_t[:, t9 * FLD:(t9 + 1) * FLD], rhs,
                        start=(t9 == 0), stop=(t9 == 8))
                ot = ot_p.tile([FLD, 512], FP16, tag="ot")
                nc.scalar.activation(ot[:MB, :], cps[:MB, :], AF.Identity,
                                     bias=cb_t[:MB, :])
                nc.scalar.activation(ot[MB:FLD, :], cps[MB:FLD, :], AF.Sigmoid,
                                     bias=cb_t[MB:FLD, :])
                for q in range(4):
                    t_in_grp = cc * 4 + q
                    ops_ = ot_pp.tile([128, FLD], FP16, tag="otps")
                    nc.tensor.transpose(ops_[:, :], ot[:, q * 128:(q + 1) * 128],
                                        id_t[:])
                    nc.scalar.activation(
                        off8[:, t_in_grp * FLD:(t_in_grp + 1) * FLD],
                        ops_[:, :], AF.Copy)
            if dbg.get("offmask") is not None:
                for t in range(GRP):
                    tg = grp * GRP + t
                    nc.sync.dma_start(
                        dbg["offmask"][tg * 128:(tg + 1) * 128, :],
                        off8[:, t * FLD:(t + 1) * FLD])

            # ---- fields ------------------------------------------------
            s_t, idx_t = build_fields(nc, tc, fp_, off8, base_c, grp, dbg)

            # ---- idx wrap: [n, (t,k)] -> [p, (t, 8k+q)] x8 replicas ----
            idq = idp.tile([128 * GRP * K], I16, tag="idq")
            nc.sync.dma_start(idq[:].rearrange("(n f) -> n f", n=128), idx_t[:])
            iqr = fp_.tile([16, 8 * GRP * K], I16, tag="iqr")
            dq = idq[:]
            dsrc = bass.AP(dq.tensor, dq.offset,
                           [[GRP * K, 16], [16 * GRP * K, 8], [1, GRP * K]])
            nc.sync.dma_start(iqr[:].rearrange("p (q f) -> p q f", q=8), dsrc)
            ixr = fp_.tile([128, GRP * K * 8], I16, tag="ixr")
            xv = ixr[:]
            dstv = bass.AP(xv.tensor, xv.offset,
                           [[xv.ap[0][0], 16], [K * 8, GRP], [8, K], [1, 8]])
            qv = iqr[:]
            srcv = bass.AP(qv.tensor, qv.offset,
                           [qv.ap[0], [K, GRP], [1, K], [GRP * K, 8]])
            nc.scalar.copy(out=dstv, in_=srcv)
            for r in range(1, 8):
                nc.sync.dma_start(ixr[r * 16:(r + 1) * 16, :], ixr[0:16, :])
            zbase = (grp // 4) * ROWS_PER_Z * ROW_EL

            # ---- per-tile gather + fold + einsum -----------------------
            for t in range(GRP):
                tl = grp * GRP + t
                g_t = gp.tile([128, K, 256], FP16, tag="G")
                sa = stag[:, :]
                in_ap = bass.AP(sa.tensor, sa.offset + zbase,
                                [[ROW_EL, 6 * ROWS_PER_Z], [1, 256]])
                nc.gpsimd.dma_gather(
                    out_ap=g_t[:], in_ap=in_ap,
                    idxs_ap=ixr[:, t * K * 8:(t + 1) * K * 8],
                    num_idxs=128 * K, num_idxs_reg=128 * K,
                    elem_size=256, elem_step=ROW_EL, single_packet=False)
                # scale: G[(k,dx), c, dzy] * s[(k,dx), -, dzy]  (3 free dims)
                sc = scp.tile([128, K * 256], FP16, tag="SC")
                g_v = bass.AP(g_t[:].tensor, g_t[:].offset,
                              [g_t[:].ap[0], [128, 2 * K], [4, C], [1, 4]])
                s_ap = s_t[:, t * K * 8:(t + 1) * K * 8]
                s_v = bass.AP(s_ap.tensor, s_ap.offset,
                              [s_ap.ap[0], [4, 2 * K], [0, C], [1, 4]])
                sc_v = bass.AP(sc[:].tensor, sc[:].offset,
                               [sc[:].ap[0], [128, 2 * K], [4, C], [1, 4]])
                nc.vector.tensor_tensor(sc_v, g_v, s_v, OP.mult)
                # fold dz only: [(k,dx), c, (dz,dy)] -> [(k,dx), c, dy];
                # dx and dy fold inside the einsum contraction
                val2 = dp.tile([128, K * 128], FP16, tag="V2")
                v2_v = bass.AP(val2[:].tensor, val2[:].offset,
                               [val2[:].ap[0], [64, 2 * K], [2, C], [1, 2]])
                j0 = bass.AP(sc[:].tensor, sc[:].offset,
                             [sc[:].ap[0], [128, 2 * K], [4, C], [1, 2]])
                j1 = bass.AP(sc[:].tensor, sc[:].offset + 2,
                             [sc[:].ap[0], [128, 2 * K], [4, C], [1, 2]])
                nc.vector.tensor_tensor(v2_v, j0, j1, OP.add)
                if dbg.get("val") is not None and tl == 0:
                    nc.sync.dma_start(dbg["val"][:], val2[:])
                # einsum: 27 chunks of (2dx, 32c, 2dy) = 128 contraction;
                # transposes land in 512-wide PSUM tiles so one ACT copy
                # covers 4 chunks
                o_ps = o_pp.tile([COUT, 128], FP32, tag="ops")
                rhs_t = rp.tile([128, NCH * 128], FP16, tag="rhs")
                for q4 in range(0, NCH, 4):
                    nq = min(4, NCH - q4)
                    tps = tp_pp.tile([128, 512], FP16, tag="tps2")
                    for qq in range(nq):
                        q = q4 + qq
                        nc.tensor.transpose(
                            tps[:, qq * 128:(qq + 1) * 128],
                            val2[:, q * 128:(q + 1) * 128], id_t[:])
                    nc.scalar.activation(
                        rhs_t[:, q4 * 128:(q4 + nq) * 128],
                        tps[:, :nq * 128], AF.Copy)
                for q in range(NCH):
                    nc.tensor.matmul(o_ps[:], we_t[:, q * COUT:(q + 1) * COUT],
                                     rhs_t[:, q * 128:(q + 1) * 128],
                                     start=(q == 0), stop=(q == NCH - 1))
                o_sb = ob_p.tile([COUT, 128], FP32, tag="osb")
                nc.scalar.activation(o_sb[:], o_ps[:], AF.Identity,
                                     bias=ob_t[:])
                nc.sync.dma_start(out_s[:, tl * 128:(tl + 1) * 128], o_sb[:])


def build_fields(nc, tc, fp_, off8, base_c, grp, dbg):
    """From off8 [128, (8t, 108)] fp16 compute s [128, (8t, 27k, 2dx, 4dzy)]
    fp16 and idx [128, (8t, 27)] int32."""
    GF = GRP * 81
    bs = fp_.tile([128, GF], FP16, tag="bs")
    nc.sync.dma_start(bs[:], base_c[:, grp * GRP:(grp + 1) * GRP, :]
                      .rearrange("p t f -> p (t f)"))
    pos = fp_.tile([128, GF], FP32, tag="pos")
    off_v = bass.AP(off8[:].tensor, off8[:].offset,
                    [off8[:].ap[0], [FLD, GRP], [1, 81]])
    bs_v = bs[:].rearrange("p (t f) -> p t f", t=GRP)
    pos_v = pos[:].rearrange("p (t f) -> p t f", t=GRP)
    nc.vector.tensor_tensor(pos_v, off_v, bs_v, OP.add)
    # floor via round-to-nearest(pos - 0.5): IEEE fp32 adds on DVE.
    # (-0.5 then +MAGIC must be two separate adds: MAGIC-0.5 is not fp32
    # representable)
    f0 = fp_.tile([128, GF], FP32, tag="f0")
    nc.vector.tensor_scalar(f0[:], pos[:], -0.5, MAGIC, OP.add, OP.add)
    nc.vector.tensor_scalar(f0[:], f0[:], -MAGIC, None, OP.add)
    # frac pair tile: [t, axis, k, 2] fp16; [...,0] = 1-frac, [...,1] = frac
    wp = fp_.tile([128, GRP * 81 * 2], FP16, tag="wp")
    wp_ap = wp[:]
    wp1 = bass.AP(wp_ap.tensor, wp_ap.offset + 1,
                  [wp_ap.ap[0], [162, GRP], [54, 3], [2, K]])
    wp0 = bass.AP(wp_ap.tensor, wp_ap.offset,
                  [wp_ap.ap[0], [162, GRP], [54, 3], [2, K]])
    pos_f = pos[:].rearrange("p (t a k) -> p t a k", t=GRP, a=3)
    f0_f = f0[:].rearrange("p (t a k) -> p t a k", t=GRP, a=3)
    nc.vector.tensor_tensor(wp1, pos_f, f0_f, OP.subtract)
    # 1 - frac = (f0 + 1) - pos
    nc.vector.scalar_tensor_tensor(wp0, f0_f, 1.0, pos_f, OP.add, OP.subtract)
    # idx = f0z*5112 + f0y*72 + f0x  (exact in fp32)
    f0z = bass.AP(f0[:].tensor, f0[:].offset, [f0[:].ap[0], [81, GRP], [1, K]])
    f0y = bass.AP(f0[:].tensor, f0[:].offset + 27,
                  [f0[:].ap[0], [81, GRP], [1, K]])
    f0x = bass.AP(f0[:].tensor, f0[:].offset + 54,
                  [f0[:].ap[0], [81, GRP], [1, K]])
    it1 = fp_.tile([128, GRP * K], FP32, tag="it1")
    it1_v = it1[:].rearrange("p (t k) -> p t k", t=GRP)
    nc.vector.tensor_scalar(it1_v, f0z, float(ROWS_PER_Z), None, OP.mult)
    nc.vector.scalar_tensor_tensor(it1_v, f0y, float(RX), it1_v, OP.mult, OP.add)
    nc.vector.tensor_tensor(it1_v, f0x, it1_v, OP.add)
    idx_t = fp_.tile([128, GRP * K], I16, tag="idx")
    nc.vector.tensor_copy(idx_t[:], it1[:])
    # s weights: [t, k, dx, dzy] = wx[dx] * wz[dz] * wy[dy] * mask
    mzy = fp_.tile([128, GRP * K * 4], FP16, tag="mzy")
    wy_v = bass.AP(wp_ap.tensor, wp_ap.offset + 54,
                   [wp_ap.ap[0], [162, GRP], [2, K], [1, 2]])
    for dz in (0, 1):  # split per dz to stay within 3 free dims
        mzy_v = bass.AP(mzy[:].tensor, mzy[:].offset + 2 * dz,
                        [mzy[:].ap[0], [108, GRP], [4, K], [1, 2]])
        wz_v = bass.AP(wp_ap.tensor, wp_ap.offset + dz,
                       [wp_ap.ap[0], [162, GRP], [2, K], [0, 2]])
        nc.vector.tensor_tensor(mzy_v, wz_v, wy_v, OP.mult)
    mk_v = bass.AP(off8[:].tensor, off8[:].offset + MB,
                   [off8[:].ap[0], [FLD, GRP], [1, K], [0, 4]])
    mzy2_v = bass.AP(mzy[:].tensor, mzy[:].offset,
                     [mzy[:].ap[0], [108, GRP], [4, K], [1, 4]])
    nc.vector.tensor_tensor(mzy2_v, mzy2_v, mk_v, OP.mult)
    s_t = fp_.tile([128, GRP * K * 8], FP16, tag="s")
    mzy3_v = bass.AP(mzy[:].tensor, mzy[:].offset,
                     [mzy[:].ap[0], [108, GRP], [4, K], [1, 4]])
    for dx in (0, 1):  # split per dx to stay within 3 free dims
        s_v = bass.AP(s_t[:].tensor, s_t[:].offset + 4 * dx,
                      [s_t[:].ap[0], [216, GRP], [8, K], [1, 4]])
        wx_v = bass.AP(wp_ap.tensor, wp_ap.offset + 108 + dx,
                       [wp_ap.ap[0], [162, GRP], [2, K], [0, 4]])
        nc.vector.tensor_tensor(s_v, wx_v, mzy3_v, OP.mult)
    if dbg.get("idx") is not None:
        for t in range(GRP):
            tg = grp * GRP + t
            nc.sync.dma_start(dbg["idx"][tg * 128:(tg + 1) * 128, :],
                              idx_t[:, t * K:(t + 1) * K])
    if dbg.get("s") is not None:
        for t in range(GRP):
            tg = grp * GRP + t
            nc.sync.dma_start(dbg["s"][tg * 128:(tg + 1) * 128, :],
                              s_t[:, t * K * 8:(t + 1) * K * 8])
    return s_t, idx_t


# ======================= host side =======================================

def _host_constants():
    kz, ky, kx = np.meshgrid(np.arange(KS), np.arange(KS), np.arange(KS),
                             indexing="ij")
    koff = np.stack([kz, ky, kx]).reshape(3, K).astype(np.float32)

    # base_c [128, NT, 81]: per tile t, partition p (voxel n = t*128+p),
    # fields (axis, k): slab-local base coords.  z base drops nz_local (the
    # gather rebases per z-slice) so indices fit int16.
    n = np.arange(N_CORE)
    ny = (n // W) % H
    nx = n % W
    bz = np.zeros_like(n)[:, None] + koff[0][None, :] + 2.0
    by = ny[:, None] + koff[1][None, :] + 3.0
    bx = nx[:, None] + koff[2][None, :] + 3.0
    base = np.concatenate([bz, by, bx], axis=1).astype(np.float16)  # [N, 81]
    base_c = base.reshape(NT, 128, 81).transpose(1, 0, 2).copy()

    ident = np.eye(128, dtype=np.float16)
    return base_c, ident


def _pack_weights(w_off, w_mask, w):
    # wconv [9, 96, 108]: for tap (kz, kx): rows (ky, c) -> cols (108 out ch)
    wcat = np.zeros((FLD, C, 3, 3, 3), np.float32)
    wcat[:81] = w_off
    wcat[MB:MB + 27] = w_mask
    wconv = np.zeros((9, 96, FLD), np.float32)
    for t9 in range(9):
        kz, kx = t9 // 3, t9 % 3
        for ky in range(3):
            wconv[t9, ky * C:(ky + 1) * C, :] = wcat[:, :, kz, ky, kx].T
    # weins [27, 128, 32]: chunk k rows (dx in 2, c in 32, dy in 2) ->
    # w[o, c, k] replicated over dx, dy
    wr = w.reshape(COUT, C, K).astype(np.float32)
    weins = np.zeros((NCH, 2, C, 2, COUT), np.float32)
    for k in range(K):
        blk = wr[:, :, k].T  # [C, COUT]
        weins[k, 0, :, 0, :] = blk
        weins[k, 0, :, 1, :] = blk
        weins[k, 1, :, 0, :] = blk
        weins[k, 1, :, 1, :] = blk
    weins = weins.reshape(NCH, 128, COUT)
    return wconv.astype(np.float16), weins.astype(np.float16)


@functools.lru_cache(maxsize=2)
def _get_module(debug_outs=False):
    return build_module(debug_outs)


def make_in_maps(inputs, debug_outs=False):
    x = np.asarray(inputs["x"], np.float32).reshape(C, D, H, W)
    base_c, ident = _host_constants()
    wconv, weins = _pack_weights(np.asarray(inputs["w_off"], np.float32),
                                 np.asarray(inputs["w_mask"], np.float32),
                                 np.asarray(inputs["w"], np.float32))
    cbias = np.zeros((FLD, 1), np.float32)
    cbias[:81, 0] = np.asarray(inputs["b_off"], np.float32)
    cbias[MB:MB + 27, 0] = np.asarray(inputs["b_mask"], np.float32)
    obias = np.asarray(inputs["b"], np.float32).reshape(COUT, 1)
    in_maps = []
    for c in range(NCORES):
        zlo = 4 * c - 3
        xs = np.zeros((C, SZ, H, W), np.float16)
        for i in range(SZ):
            z = zlo + i
            if 0 <= z < D:
                xs[:, i] = x[:, z].astype(np.float16)
        # staging rows: row (z0,y0,x0) = xpad[c, z0+dz, y0+dy, x0], [c][dzy]
        xpad = np.zeros((C, SZ, RY + 1, RX), np.float16)
        xpad[:, :, 4:4 + H, 4:4 + W] = xs
        stagv = np.zeros((SZ, RY, RX, C, 4), np.float16)
        for dz in (0, 1):
            for dy in (0, 1):
                zn = SZ - dz
                stagv[:zn, :, :, :, dz * 2 + dy] = np.transpose(
                    xpad[:, dz:dz + zn, dy:dy + RY, :], (1, 2, 3, 0))
        stag = stagv.reshape(NROWS, ROW_EL)
        # conv slab, (ky, c) replicated, pad-1 in x, clipped z
        xr_h = np.zeros((96, CZ, CY, CX), np.float16)
        for g in range(3):
            ylo, yhi = max(0, 1 - g), min(H, H + 1 - g)
            xr_h[g * C:(g + 1) * C, :, ylo:yhi, 1:1 + W] = \
                xs[:, 2:2 + CZ, ylo + g - 1:yhi + g - 1, :]
        in_maps.append({
            "xs_conv": xr_h.reshape(96, -1), "stag": stag, "base_c": base_c,
            "wconv": wconv, "weins": weins, "cbias": cbias, "obias": obias,
            "ident": ident,
        })
    return in_maps


def kernel(**inputs):
    nc = _get_module(False)
    in_maps = make_in_maps(inputs)
    res = run_bass_kernel_spmd(nc, in_maps, core_ids=list(range(NCORES)))
    out = np.empty((1, COUT, D, H, W), np.float32)
    for c in range(NCORES):
        out[0, :, 4 * c:4 * (c + 1)] = (
            res.results[c]["out_s"].reshape(COUT, ZB, H, W))
    return out



# revision 10
# speedup vs baseline: 2.0584x; 2.0584x over previous
"""Deformable 3D conv (nn_DeformableConv3D) Trainium2 Bass kernel.

Strategy (per core, 8 cores, sharded over output z in blocks of 4):
  1. offset/mask 3x3x3 convs as implicit GEMM: contraction packed (ky,c)=96,
     9 matmuls per 512-voxel chunk accumulating in PSUM.
  2. Per 128-voxel tile, transpose offset/mask fields to [n, ch] layout,
     compute trilinear floors/fracs/corner-weights and int32 gather indices
     on DVE/ACT.
  3. Build a DRAM staging buffer of (z,y)-corner-packed fp16 rows
     (row m=(z0,y0,x0) holds [c=32][dzy=4]; x-corner pair = adjacent rows),
     gather 512B elems (8 corners x 32ch) per (tap, voxel) sample with
     indirect DMA.
  4. Scale by (mask * trilinear) weights + fold corners on DVE; last corner
     pair folds into the main einsum contraction.
  5. Main einsum as 14 matmul chunks of contraction (2 taps x 32c x 2 pair),
     PE transposes to get [contraction, n] tiles.
"""
import os
import sys
import functools

import numpy as np

for _p in ("/opt/trn_rl_repo", "/root/.axon_site/_ro/trn_rl_repo"):
    if os.path.isdir(_p) and _p not in sys.path:
        sys.path.insert(0, _p)

from concourse import bass, bacc, mybir  # noqa: E402
import concourse.tile as tile  # noqa: E402
from concourse.bass_utils import run_bass_kernel_spmd  # noqa: E402

FP32 = mybir.dt.float32
FP16 = mybir.dt.float16
I32 = mybir.dt.int32
I16 = mybir.dt.int16
AF = mybir.ActivationFunctionType
OP = mybir.AluOpType

# problem constants
C, D, H, W = 32, 32, 64, 64
KS, K, COUT = 3, 27, 32
NCORES = 8
ZB = D // NCORES                # output z per core (4)
N_CORE = ZB * H * W             # 16384 voxels per core
NT = N_CORE // 128              # 128 n-tiles per core
GRP = 8                         # n-tiles per field group
NG = NT // GRP                  # 16 groups

# staging geometry: slab-local padded coords
# pos_z_local = nz_local + kz + 2 + off   (in [0.15, 8.85])
# pos_y_local = ny + ky + 3 + off         (in [1.15, 69.85])
# pos_x_local = nx + kx + 3 + off
SZ = 10                         # z slices in x slab (z in [4c-3, 4c+7))
RY = H + 7                      # 71 y0 rows
RX = W + 8                      # 72 x values per row
ROWS_PER_Z = RY * RX            # 5112
NROWS = SZ * ROWS_PER_Z         # 51120 staging rows
ROW_EL = C * 4                  # 128 fp16 per row (c x dzy)

# conv slab geometry: z in [4c-1, 4c+5) -> 6 slices, x margin 1 -> 66
CZ, CY, CX = 6, H, W + 2

# magic constants for floor-by-round-to-nearest (fp32)
MAGIC = 12582912.0              # 1.5 * 2**23

FLD = 128                       # 0:81 offsets, 96:123 mask, rest pad
MB = 96                         # mask block start
NQ = 7                          # einsum chunks of 128 (k,c) rows; 864 real


def build_module(debug_outs=False):
    nc = bacc.Bacc("TRN2", target_bir_lowering=False, debug=False,
                   num_swdge_queues=4)

    # ---- dram I/O -------------------------------------------------------
    xs_conv = nc.dram_tensor("xs_conv", [96, CZ * CY * CX], FP16,
                             kind="ExternalInput")
    base_c = nc.dram_tensor("base_c", [128, NT, 81], FP16, kind="ExternalInput")
    wconv = nc.dram_tensor("wconv", [9, 96, FLD], FP16, kind="ExternalInput")
    weins = nc.dram_tensor("weins", [NQ, 128, COUT], FP16, kind="ExternalInput")
    cbias = nc.dram_tensor("cbias", [FLD, 1], FP32, kind="ExternalInput")
    obias = nc.dram_tensor("obias", [COUT, 1], FP32, kind="ExternalInput")
    ident = nc.dram_tensor("ident", [128, 128], FP16, kind="ExternalInput")
    out_s = nc.dram_tensor("out_s", [COUT, N_CORE], FP32, kind="ExternalOutput")
    stag = nc.dram_tensor("stag", [NROWS, ROW_EL], FP16, kind="ExternalInput")

    dbg = {}
    if debug_outs:
        dbg["offmask"] = nc.dram_tensor("d_offmask", [N_CORE, FLD], FP16,
                                        kind="ExternalOutput")
        dbg["idx"] = nc.dram_tensor("d_idx", [N_CORE, K], I16,
                                    kind="ExternalOutput")
        dbg["s"] = nc.dram_tensor("d_s", [N_CORE, K * 8], FP16,
                                  kind="ExternalOutput")
        dbg["val"] = nc.dram_tensor("d_val", [128, K * 64], FP16,
                                    kind="ExternalOutput")

    with tile.TileContext(nc) as tc:
        build_kernel(nc, tc, xs_conv, base_c, wconv, weins, cbias, obias,
                     ident, out_s, stag, dbg)
    nc.compile()
    return nc


def build_kernel(nc, tc, xs_conv, base_c, wconv, weins, cbias, obias,
                 ident, out_s, stag, dbg):
    with tc.tile_pool(name="konst", bufs=1) as kp:
        id_t = kp.tile([128, 128], FP16)
        nc.sync.dma_start(id_t[:], ident[:])
        cb_t = kp.tile([FLD, 1], FP32)
        nc.sync.dma_start(cb_t[:], cbias[:])
        ob_t = kp.tile([COUT, 1], FP32)
        nc.sync.dma_start(ob_t[:], obias[:])
        wc_t = kp.tile([96, 9 * FLD], FP16, tag="wc")
        for t9 in range(9):
            nc.sync.dma_start(wc_t[:, t9 * FLD:(t9 + 1) * FLD],
                              wconv[t9, :, :])
        we_t = kp.tile([128, NQ * COUT], FP16, tag="we")
        for q in range(NQ):
            nc.sync.dma_start(we_t[:, q * COUT:(q + 1) * COUT],
                              weins[q, :, :])

        with tc.tile_pool(name="xr", bufs=1) as xrp:
            xr = xrp.tile([96, CZ * CY * CX], FP16)  # (ky,c) x (z,y,x)
            nc.sync.dma_start(xr[:], xs_conv[:])
            xr_v = xr[:].rearrange("p (z y x) -> p z y x", z=CZ, y=CY, x=CX)
            main_loop(nc, tc, xr_v, base_c, wc_t, we_t, cb_t, ob_t, id_t,
                      stag, out_s, dbg)


def main_loop(nc, tc, xr_v, base_c, wc_t, we_t, cb_t, ob_t, id_t, stag,
              out_s, dbg):
    with (
        tc.tile_pool(name="cps", bufs=2, space="PSUM") as conv_pp,
        tc.tile_pool(name="ot", bufs=2) as ot_p,
        tc.tile_pool(name="otps", bufs=2, space="PSUM") as ot_pp,
        tc.tile_pool(name="fld", bufs=2) as fp_,
        tc.tile_pool(name="idxd", bufs=2, space="DRAM") as idp,
        tc.tile_pool(name="gat", bufs=5) as gp,
        tc.tile_pool(name="fold", bufs=2) as dp,
        tc.tile_pool(name="tps", bufs=2, space="PSUM") as tp_pp,
        tc.tile_pool(name="rhs", bufs=2) as rp,
        tc.tile_pool(name="ops", bufs=2, space="PSUM") as o_pp,
        tc.tile_pool(name="osb", bufs=2) as ob_p,
    ):
        for grp in range(NG):
            # ---- conv: 2 chunks of 512 voxels --------------------------
            off8 = fp_.tile([128, GRP * FLD], FP16, tag="off8")
            for cc in range(2):
                chunk = grp * 2 + cc          # global 512-chunk id
                zl = chunk // 8               # local out z
                y0 = (chunk % 8) * 8          # first of 8 y rows
                cps = conv_pp.tile([FLD, 512], FP32, tag="cpsum")
                for t9 in range(9):
                    kz, kx = t9 // 3, t9 % 3
                    rhs = xr_v[:, zl + kz, y0:y0 + 8, kx:kx + W]
                    nc.tensor.matmul(
                        cps[:], wc_t[:, t9 * FLD:(t9 + 1) * FLD], rhs,
                        start=(t9 == 0), stop=(t9 == 8))
                ot = ot_p.tile([FLD, 512], FP16, tag="ot")
                nc.scalar.activation(ot[:MB, :], cps[:MB, :], AF.Identity,
                                     bias=cb_t[:MB, :])
                nc.scalar.activation(ot[MB:FLD, :], cps[MB:FLD, :], AF.Sigmoid,
                                     bias=cb_t[MB:FLD, :])
                for q in range(4):
                    t_in_grp = cc * 4 + q
                    ops_ = ot_pp.tile([128, FLD], FP16, tag="otps")
                    nc.tensor.transpose(ops_[:, :], ot[:, q * 128:(q + 1) * 128],
                                        id_t[:])
                    nc.scalar.activation(
                        off8[:, t_in_grp * FLD:(t_in_grp + 1) * FLD],
                        ops_[:, :], AF.Copy)
            if dbg.get("offmask") is not None:
                for t in range(GRP):
                    tg = grp * GRP + t
                    nc.sync.dma_start(
                        dbg["offmask"][tg * 128:(tg + 1) * 128, :],
                        off8[:, t * FLD:(t + 1) * FLD])

            # ---- fields ------------------------------------------------
            s_t, idx_t = build_fields(nc, tc, fp_, off8, base_c, grp, dbg)

            # ---- idx wrap: [n, (t,k)] -> [p, (t, 8k+q)] x8 replicas ----
            idq = idp.tile([128 * GRP * K], I16, tag="idq")
            nc.sync.dma_start(idq[:].rearrange("(n f) -> n f", n=128), idx_t[:])
            iqr = fp_.tile([16, 8 * GRP * K], I16, tag="iqr")
            dq = idq[:]
            dsrc = bass.AP(dq.tensor, dq.offset,
                           [[GRP * K, 16], [16 * GRP * K, 8], [1, GRP * K]])
            nc.sync.dma_start(iqr[:].rearrange("p (q f) -> p q f", q=8), dsrc)
            ixr = fp_.tile([128, GRP * K * 8], I16, tag="ixr")
            xv = ixr[:]
            dstv = bass.AP(xv.tensor, xv.offset,
                           [[xv.ap[0][0], 16], [K * 8, GRP], [8, K], [1, 8]])
            qv = iqr[:]
            srcv = bass.AP(qv.tensor, qv.offset,
                           [qv.ap[0], [K, GRP], [1, K], [GRP * K, 8]])
            nc.scalar.copy(out=dstv, in_=srcv)
            for r in range(1, 8):
                nc.sync.dma_start(ixr[r * 16:(r + 1) * 16, :], ixr[0:16, :])
            zbase = (grp // 4) * ROWS_PER_Z * ROW_EL

            # ---- per-tile gather + fold + einsum -----------------------
            for t in range(GRP):
                tl = grp * GRP + t
                g_t = gp.tile([128, K, 256], FP16, tag="G")
                sa = stag[:, :]
                in_ap = bass.AP(sa.tensor, sa.offset + zbase,
                                [[ROW_EL, 6 * ROWS_PER_Z], [1, 256]])
                nc.gpsimd.dma_gather(
                    out_ap=g_t[:], in_ap=in_ap,
                    idxs_ap=ixr[:, t * K * 8:(t + 1) * K * 8],
                    num_idxs=128 * K, num_idxs_reg=128 * K,
                    elem_size=256, elem_step=ROW_EL, single_packet=False,
                    queue_num=t % 4)
                # scale in place: G[(k,dx), c, dzy] *= s[(k,dx), -, dzy]
                g_v = bass.AP(g_t[:].tensor, g_t[:].offset,
                              [g_t[:].ap[0], [128, 2 * K], [4, C], [1, 4]])
                s_ap = s_t[:, t * K * 8:(t + 1) * K * 8]
                s_v = bass.AP(s_ap.tensor, s_ap.offset,
                              [s_ap.ap[0], [4, 2 * K], [0, C], [1, 4]])
                nc.vector.tensor_tensor(g_v, g_v, s_v, OP.mult)
                # fold dz: [(k,dx), c, (dz,dy)] -> [(k,dx), c, dy]
                val2 = dp.tile([128, K * 128], FP16, tag="V2")
                v2_v = bass.AP(val2[:].tensor, val2[:].offset,
                               [val2[:].ap[0], [64, 2 * K], [2, C], [1, 2]])
                j0 = bass.AP(g_t[:].tensor, g_t[:].offset,
                             [g_t[:].ap[0], [128, 2 * K], [4, C], [1, 2]])
                j1 = bass.AP(g_t[:].tensor, g_t[:].offset + 2,
                             [g_t[:].ap[0], [128, 2 * K], [4, C], [1, 2]])
                nc.vector.tensor_tensor(v2_v, j0, j1, OP.add)
                # fold dy: [(k,dx), c, dy] -> [(k,dx), c]
                v25 = dp.tile([128, K * 64], FP16, tag="V25")
                v25_v = bass.AP(v25[:].tensor, v25[:].offset,
                                [v25[:].ap[0], [32, 2 * K], [1, C]])
                y0_ = bass.AP(val2[:].tensor, val2[:].offset,
                              [val2[:].ap[0], [64, 2 * K], [2, C]])
                y1_ = bass.AP(val2[:].tensor, val2[:].offset + 1,
                              [val2[:].ap[0], [64, 2 * K], [2, C]])
                nc.vector.tensor_tensor(v25_v, y0_, y1_, OP.add)
                # fold dx: [k, (dx), c] -> [k, c]; 864 contraction rows
                val3 = dp.tile([128, NQ * 128], FP16, tag="V3")
                nc.vector.memset(val3[:, K * C:], 0.0)
                v3_v = bass.AP(val3[:].tensor, val3[:].offset,
                               [val3[:].ap[0], [C, K], [1, C]])
                x0_ = bass.AP(v25[:].tensor, v25[:].offset,
                              [v25[:].ap[0], [64, K], [1, C]])
                x1_ = bass.AP(v25[:].tensor, v25[:].offset + 32,
                              [v25[:].ap[0], [64, K], [1, C]])
                nc.vector.tensor_tensor(v3_v, x0_, x1_, OP.add)
                if dbg.get("val") is not None and tl == 0:
                    nc.sync.dma_start(dbg["val"][:], val3[:])
                # einsum: 7 chunks of 128 (k,c) rows (rows >= 864 are
                # garbage x zero weights)
                o_ps = o_pp.tile([COUT, 128], FP32, tag="ops")
                rhs_t = rp.tile([128, NQ * 128], FP16, tag="rhs")
                tps = tp_pp.tile([128, NQ * 128], FP16, tag="tps2")
                for q in range(NQ):
                    nc.tensor.transpose(
                        tps[:, q * 128:(q + 1) * 128],
                        val3[:, q * 128:(q + 1) * 128], id_t[:])
                nc.scalar.activation(rhs_t[:], tps[:], AF.Copy)
                for q in range(NQ):
                    nc.tensor.matmul(o_ps[:], we_t[:, q * COUT:(q + 1) * COUT],
                                     rhs_t[:, q * 128:(q + 1) * 128],
                                     start=(q == 0), stop=(q == NQ - 1))
                o_sb = ob_p.tile([COUT, 128], FP32, tag="osb")
                nc.scalar.activation(o_sb[:], o_ps[:], AF.Identity,
                                     bias=ob_t[:])
                nc.sync.dma_start(out_s[:, tl * 128:(tl + 1) * 128], o_sb[:])


def build_fields(nc, tc, fp_, off8, base_c, grp, dbg):
    """From off8 [128, (8t, 108)] fp16 compute s [128, (8t, 27k, 2dx, 4dzy)]
    fp16 and idx [128, (8t, 27)] int32."""
    GF = GRP * 81
    bs = fp_.tile([128, GF], FP16, tag="bs")
    nc.sync.dma_start(bs[:], base_c[:, grp * GRP:(grp + 1) * GRP, :]
                      .rearrange("p t f -> p (t f)"))
    pos = fp_.tile([128, GF], FP32, tag="pos")
    off_v = bass.AP(off8[:].tensor, off8[:].offset,
                    [off8[:].ap[0], [FLD, GRP], [1, 81]])
    bs_v = bs[:].rearrange("p (t f) -> p t f", t=GRP)
    pos_v = pos[:].rearrange("p (t f) -> p t f", t=GRP)
    nc.vector.tensor_tensor(pos_v, off_v, bs_v, OP.add)
    # floor via round-to-nearest(pos - 0.5): IEEE fp32 adds on DVE.
    # (-0.5 then +MAGIC must be two separate adds: MAGIC-0.5 is not fp32
    # representable)
    f0 = fp_.tile([128, GF], FP32, tag="f0")
    nc.vector.tensor_scalar(f0[:], pos[:], -0.5, MAGIC, OP.add, OP.add)
    nc.vector.tensor_scalar(f0[:], f0[:], -MAGIC, None, OP.add)
    # frac pair tile: [t, axis, k, 2] fp16; [...,0] = 1-frac, [...,1] = frac
    wp = fp_.tile([128, GRP * 81 * 2], FP16, tag="wp")
    wp_ap = wp[:]
    wp1 = bass.AP(wp_ap.tensor, wp_ap.offset + 1,
                  [wp_ap.ap[0], [162, GRP], [54, 3], [2, K]])
    wp0 = bass.AP(wp_ap.tensor, wp_ap.offset,
                  [wp_ap.ap[0], [162, GRP], [54, 3], [2, K]])
    pos_f = pos[:].rearrange("p (t a k) -> p t a k", t=GRP, a=3)
    f0_f = f0[:].rearrange("p (t a k) -> p t a k", t=GRP, a=3)
    nc.vector.tensor_tensor(wp1, pos_f, f0_f, OP.subtract)
    # 1 - frac = (f0 + 1) - pos
    nc.vector.scalar_tensor_tensor(wp0, f0_f, 1.0, pos_f, OP.add, OP.subtract)
    # idx = f0z*5112 + f0y*72 + f0x  (exact in fp32)
    f0z = bass.AP(f0[:].tensor, f0[:].offset, [f0[:].ap[0], [81, GRP], [1, K]])
    f0y = bass.AP(f0[:].tensor, f0[:].offset + 27,
                  [f0[:].ap[0], [81, GRP], [1, K]])
    f0x = bass.AP(f0[:].tensor, f0[:].offset + 54,
                  [f0[:].ap[0], [81, GRP], [1, K]])
    it1 = fp_.tile([128, GRP * K], FP32, tag="it1")
    it1_v = it1[:].rearrange("p (t k) -> p t k", t=GRP)
    nc.vector.tensor_scalar(it1_v, f0z, float(ROWS_PER_Z), None, OP.mult)
    nc.vector.scalar_tensor_tensor(it1_v, f0y, float(RX), it1_v, OP.mult, OP.add)
    nc.vector.tensor_tensor(it1_v, f0x, it1_v, OP.add)
    idx_t = fp_.tile([128, GRP * K], I16, tag="idx")
    nc.vector.tensor_copy(idx_t[:], it1[:])
    # s weights: [t, k, dx, dzy] = wx[dx] * wz[dz] * wy[dy] * mask
    mzy = fp_.tile([128, GRP * K * 4], FP16, tag="mzy")
    wy_v = bass.AP(wp_ap.tensor, wp_ap.offset + 54,
                   [wp_ap.ap[0], [162, GRP], [2, K], [1, 2]])
    for dz in (0, 1):  # split per dz to stay within 3 free dims
        mzy_v = bass.AP(mzy[:].tensor, mzy[:].offset + 2 * dz,
                        [mzy[:].ap[0], [108, GRP], [4, K], [1, 2]])
        wz_v = bass.AP(wp_ap.tensor, wp_ap.offset + dz,
                       [wp_ap.ap[0], [162, GRP], [2, K], [0, 2]])
        nc.vector.tensor_tensor(mzy_v, wz_v, wy_v, OP.mult)
    mk_v = bass.AP(off8[:].tensor, off8[:].offset + MB,
                   [off8[:].ap[0], [FLD, GRP], [1, K], [0, 4]])
    mzy2_v = bass.AP(mzy[:].tensor, mzy[:].offset,
                     [mzy[:].ap[0], [108, GRP], [4, K], [1, 4]])
    nc.vector.tensor_tensor(mzy2_v, mzy2_v, mk_v, OP.mult)
    s_t = fp_.tile([128, GRP * K * 8], FP16, tag="s")
    mzy3_v = bass.AP(mzy[:].tensor, mzy[:].offset,
                     [mzy[:].ap[0], [108, GRP], [4, K], [1, 4]])
    for dx in (0, 1):  # split per dx to stay within 3 free dims
        s_v = bass.AP(s_t[:].tensor, s_t[:].offset + 4 * dx,
                      [s_t[:].ap[0], [216, GRP], [8, K], [1, 4]])
        wx_v = bass.AP(wp_ap.tensor, wp_ap.offset + 108 + dx,
                       [wp_ap.ap[0], [162, GRP], [2, K], [0, 4]])
        nc.vector.tensor_tensor(s_v, wx_v, mzy3_v, OP.mult)
    if dbg.get("idx") is not None:
        for t in range(GRP):
            tg = grp * GRP + t
            nc.sync.dma_start(dbg["idx"][tg * 128:(tg + 1) * 128, :],
                              idx_t[:, t * K:(t + 1) * K])
    if dbg.get("s") is not None:
        for t in range(GRP):
            tg = grp * GRP + t
            nc.sync.dma_start(dbg["s"][tg * 128:(tg + 1) * 128, :],
                              s_t[:, t * K * 8:(t + 1) * K * 8])
    return s_t, idx_t


# ======================= host side =======================================

def _host_constants():
    kz, ky, kx = np.meshgrid(np.arange(KS), np.arange(KS), np.arange(KS),
                             indexing="ij")
    koff = np.stack([kz, ky, kx]).reshape(3, K).astype(np.float32)

    # base_c [128, NT, 81]: per tile t, partition p (voxel n = t*128+p),
    # fields (axis, k): slab-local base coords.  z base drops nz_local (the
    # gather rebases per z-slice) so indices fit int16.
    n = np.arange(N_CORE)
    ny = (n // W) % H
    nx = n % W
    bz = np.zeros_like(n)[:, None] + koff[0][None, :] + 2.0
    by = ny[:, None] + koff[1][None, :] + 3.0
    bx = nx[:, None] + koff[2][None, :] + 3.0
    base = np.concatenate([bz, by, bx], axis=1).astype(np.float16)  # [N, 81]
    base_c = base.reshape(NT, 128, 81).transpose(1, 0, 2).copy()

    ident = np.eye(128, dtype=np.float16)
    return base_c, ident


def _pack_weights(w_off, w_mask, w):
    # wconv [9, 96, 108]: for tap (kz, kx): rows (ky, c) -> cols (108 out ch)
    wcat = np.zeros((FLD, C, 3, 3, 3), np.float32)
    wcat[:81] = w_off
    wcat[MB:MB + 27] = w_mask
    wconv = np.zeros((9, 96, FLD), np.float32)
    for t9 in range(9):
        kz, kx = t9 // 3, t9 % 3
        for ky in range(3):
            wconv[t9, ky * C:(ky + 1) * C, :] = wcat[:, :, kz, ky, kx].T
    # weins [7, 128, 32]: contraction row r = k*32 + c -> w[o, c, k];
    # rows >= 864 zero (they meet garbage rhs rows)
    wr = w.reshape(COUT, C, K).astype(np.float32)
    weins = np.zeros((NQ * 128, COUT), np.float32)
    weins[:K * C] = wr.transpose(2, 1, 0).reshape(K * C, COUT)
    weins = weins.reshape(NQ, 128, COUT)
    return wconv.astype(np.float16), weins.astype(np.float16)


@functools.lru_cache(maxsize=2)
def _get_module(debug_outs=False):
    return build_module(debug_outs)


def make_in_maps(inputs, debug_outs=False):
    x = np.asarray(inputs["x"], np.float32).reshape(C, D, H, W)
    base_c, ident = _host_constants()
    wconv, weins = _pack_weights(np.asarray(inputs["w_off"], np.float32),
                                 np.asarray(inputs["w_mask"], np.float32),
                                 np.asarray(inputs["w"], np.float32))
    cbias = np.zeros((FLD, 1), np.float32)
    cbias[:81, 0] = np.asarray(inputs["b_off"], np.float32)
    cbias[MB:MB + 27, 0] = np.asarray(inputs["b_mask"], np.float32)
    obias = np.asarray(inputs["b"], np.float32).reshape(COUT, 1)
    in_maps = []
    for c in range(NCORES):
        zlo = 4 * c - 3
        xs = np.zeros((C, SZ, H, W), np.float16)
        for i in range(SZ):
            z = zlo + i
            if 0 <= z < D:
                xs[:, i] = x[:, z].astype(np.float16)
        # staging rows: row (z0,y0,x0) = xpad[c, z0+dz, y0+dy, x0], [c][dzy]
        xpad = np.zeros((C, SZ, RY + 1, RX), np.float16)
        xpad[:, :, 4:4 + H, 4:4 + W] = xs
        stagv = np.zeros((SZ, RY, RX, C, 4), np.float16)
        for dz in (0, 1):
            for dy in (0, 1):
                zn = SZ - dz
                stagv[:zn, :, :, :, dz * 2 + dy] = np.transpose(
                    xpad[:, dz:dz + zn, dy:dy + RY, :], (1, 2, 3, 0))
        stag = stagv.reshape(NROWS, ROW_EL)
        # conv slab, (ky, c) replicated, pad-1 in x, clipped z
        xr_h = np.zeros((96, CZ, CY, CX), np.float16)
        for g in range(3):
            ylo, yhi = max(0, 1 - g), min(H, H + 1 - g)
            xr_h[g * C:(g + 1) * C, :, ylo:yhi, 1:1 + W] = \
                xs[:, 2:2 + CZ, ylo + g - 1:yhi + g - 1, :]
        in_maps.append({
            "xs_conv": xr_h.reshape(96, -1), "stag": stag, "base_c": base_c,
            "wconv": wconv, "weins": weins, "cbias": cbias, "obias": obias,
            "ident": ident,
        })
    return in_maps


def kernel(**inputs):
    nc = _get_module(False)
    in_maps = make_in_maps(inputs)
    res = run_bass_kernel_spmd(nc, in_maps, core_ids=list(range(NCORES)))
    out = np.empty((1, COUT, D, H, W), np.float32)
    for c in range(NCORES):
        out[0, :, 4 * c:4 * (c + 1)] = (
            res.results[c]["out_s"].reshape(COUT, ZB, H, W))
    return out

